# revision 4
# baseline (speedup 1.0000x reference)
"""Trainium2 Bass kernel for nn_CustomReshapeLayer (staircase sliding-window gather).

reference: out[b, i, j] = inputs[b, start[i] + j], start[i] = i*M - i*(i-1)//2,
shapes: inputs [32, 2098176] f32 -> out [32, 2048, 2048] f32. jnp.take's default
out-of-bounds mode fills with NaN; rows i >= ~1985 run past the end of the input,
so those positions must be NaN.

Design (data-parallel over 8 NeuronCores, 4 batch rows per core):
  - Host pads each batch row with M NaNs so out-of-bounds window tails read NaN.
  - Per 128-row block: SWDGE indirect DMA gathers 128 windows (one int32 element
    offset per partition, 8KB contiguous per window) HBM -> SBUF, then a single
    HWDGE DMA stores the [128, 2048] tile to its contiguous slot in the output.
  - Raw-Bass two-engine pipeline (gpsimd gathers / sync stores), NBUF rotating
    buffers + per-buffer semaphores. Block(no_gpsimd_drain=True) skips the
    ~35us SWDGE drain at kernel exit (all gathers are provably complete: their
    completion semaphores gate the stores, and every store is waited on).
"""

from contextlib import ExitStack

import numpy as np

import concourse.bass as bass
import concourse.mybir as mybir
from concourse.bass_utils import run_bass_kernel_spmd

M = 2048
VEC = M * (M + 1) // 2  # 2,098,176
VECP = VEC + M  # per-batch stride incl. NaN pad
B_FULL = 32
NCORES = 8
B_CORE = B_FULL // NCORES  # 4
NPAIR = M // 256  # 256-row pair-blocks per batch = 8
NPAIRS = B_CORE * NPAIR  # 32 stores, 64 gathers
NBUF = 4  # rotating [128, 2*M] buffers

_cache: dict = {}


def _starts() -> np.ndarray:
    i = np.arange(M, dtype=np.int64)
    return i * M - (i * (i - 1)) // 2


def _make_indices() -> np.ndarray:
    """int32 [128, 2*NPAIRS]; col 2j+h holds start[r0 + 2p + h] + b*VECP, so
    partition p gathers rows 2p and 2p+1 of its 256-row block adjacently and
    the store becomes one fully contiguous 2MB HBM write."""
    starts = _starts()
    idx = np.empty((128, 2 * NPAIRS), dtype=np.int32)
    p = np.arange(128)
    for j in range(NPAIRS):
        b, blk = divmod(j, NPAIR)
        r0 = blk * 256
        for h in range(2):
            idx[:, 2 * j + h] = (b * VECP + starts[r0 + 2 * p + h]).astype(np.int32)
    return idx


def _pad_input(x_core: np.ndarray) -> np.ndarray:
    """[B_CORE, VEC] -> flat [B_CORE*VECP, 1]; pad reads as jnp.take's NaN fill."""
    out = np.full((B_CORE, VECP), np.nan, dtype=np.float32)
    out[:, :VEC] = x_core
    return out.reshape(-1, 1)


def _build_nc() -> bass.Bass:
    nc = bass.Bass()
    x = nc.declare_dram_parameter(
        "x", [B_CORE * VECP, 1], mybir.dt.float32, isOutput=False
    )
    idx = nc.declare_dram_parameter(
        "idx", [128, 2 * NPAIRS], mybir.dt.int32, isOutput=False
    )
    y = nc.declare_dram_parameter("y", [B_CORE, M, M], mybir.dt.float32, isOutput=True)

    with ExitStack() as stack:
        idx_sb = stack.enter_context(
            nc.sbuf_tensor("idx_sb", [128, 2 * NPAIRS], mybir.dt.int32)
        )
        bufs = [
            stack.enter_context(
                nc.sbuf_tensor(f"buf{i}", [128, 2 * M], mybir.dt.float32)
            )
            for i in range(NBUF)
        ]
        idx_sem = stack.enter_context(nc.semaphore("idx_sem"))
        ga_sems = [stack.enter_context(nc.semaphore(f"ga{i}")) for i in range(NBUF)]
        gb_sems = [stack.enter_context(nc.semaphore(f"gb{i}")) for i in range(NBUF)]
        st_sems = [stack.enter_context(nc.semaphore(f"st{i}")) for i in range(NBUF)]
        block = stack.enter_context(nc.Block(no_gpsimd_drain=True))

        @block.gpsimd
        def _(gpsimd):
            gpsimd.dma_start(out=idx_sb[:], in_=idx[:]).then_inc(idx_sem, 16)
            gpsimd.wait_ge(idx_sem, 16)
            for j in range(NPAIRS):
                k = j % NBUF
                if j >= NBUF:
                    gpsimd.wait_ge(st_sems[k], 16 * (j // NBUF))
                for h, sems in ((0, ga_sems), (1, gb_sems)):
                    gpsimd.indirect_dma_start(
                        out=bufs[k][:, h * M : (h + 1) * M],
                        out_offset=None,
                        in_=x[:],
                        in_offset=bass.IndirectOffsetOnAxis(
                            ap=idx_sb[:, 2 * j + h : 2 * j + h + 1], axis=0
                        ),
                    ).then_inc(sems[k], 16)

        @block.sync
        def _(sync):
            for j in range(NPAIRS):
                k = j % NBUF
                sync.wait_ge(ga_sems[k], 16 * (j // NBUF + 1))
                sync.wait_ge(gb_sems[k], 16 * (j // NBUF + 1))
                b, blk = divmod(j, NPAIR)
                r0 = blk * 256
                sync.dma_start(
                    out=y[b, r0 : r0 + 256, :].rearrange(
                        "(p h) m -> p (h m)", h=2
                    ),
                    in_=bufs[k][:],
                ).then_inc(st_sems[k], 16)
            for k in range(NBUF):
                sync.wait_ge(st_sems[k], 16 * (NPAIRS // NBUF))
    return nc


def _run(inputs: np.ndarray, trace: bool = False):
    """inputs [32, VEC] f32 -> (out [32, M, M] f32, exec_time_ns | None)."""
    assert inputs.shape == (B_FULL, VEC), inputs.shape
    x = np.ascontiguousarray(inputs, dtype=np.float32)
    if "nc" not in _cache:
        _cache["nc"] = _build_nc()
        _cache["idx"] = _make_indices()
    nc, idx = _cache["nc"], _cache["idx"]
    in_maps = [
        {"x": _pad_input(x[c * B_CORE : (c + 1) * B_CORE]), "idx": idx}
        for c in range(NCORES)
    ]
    res = run_bass_kernel_spmd(nc, in_maps, list(range(NCORES)), trace=trace)
    out = np.concatenate([res.results[c]["y"] for c in range(NCORES)], axis=0)
    return out, res.exec_time_ns


def kernel(inputs: np.ndarray) -> np.ndarray:
    out, _ = _run(np.asarray(inputs))
    return out
